# revision 18
# baseline (speedup 1.0000x reference)
"""MoE layer (dense routing, 8 experts) on 8 TRN2 NeuronCores.

Expert-parallel: core e owns expert e; x and the gating network are
replicated.  Core e computes y_e^T = g_e * (relu(x W1[e] + b1) W2[e] + b2)^T
and the host sums the 8 per-core outputs and transposes once.

V2 layout: both GEMMs are weight-stationary and every stationary load is
shared by G=2 batch tiles (1024 batch columns in flight):

  - GEMM1 (transposed): hT[n, b] = relu(sum_d W1[d, n] xT[d, b] + b1[n]),
    stationary = W1 128x128 blocks (streamed from HBM per nt-block in a
    small rotating buffer), moving = xT batch tiles.  Each stationary block
    feeds the G batch tiles before switching.
  - GEMM2 (transposed output): yT[o, b] = sum_h W2[h, o] hT[h, b],
    stationary = W2 128x128 blocks (fully resident), moving = the hT tiles
    produced by GEMM1.  Epilogue fuses PSUM evacuation with bias and gate:
    y_sbuf = (psum + b2[o]) * gate_bcast[b].  Output stays transposed in
    DRAM; the host untransposes after the cross-core sum.

  - Gate: logitsT[e, b] for all 8 experts via 8 small matmuls, with the
    expert axis permuted per core so the own expert is row 0; exp on
    ScalarE with fused bg bias; softmax denominator via GPSIMD partition
    all-reduce; gate row = exp[0] * 1/den (DVE); broadcast to 128
    partitions on GPSIMD.  Beyond the logits the gate never touches the
    PE (no transposes, no num/den/broadcast matmuls).

All matmuls bf16 x bf16 -> fp32 PSUM.  W2 lives in SBUF for the whole
kernel; W1 streams per (group, nt) block; x streams in 512-column tiles.
"""

import numpy as np
import ml_dtypes

import concourse.bacc as bacc
import concourse.mybir as mybir
import concourse.tile as tile
from concourse.bass_isa import ReduceOp
from concourse.bass_utils import run_bass_kernel_spmd

B, D_IN, D_HID, D_OUT, E = 8192, 1024, 4096, 1024, 8
NCORES = 8
BT = 512                 # batch tile (matmul moving free dim)
G = 2                    # batch tiles sharing each stationary load
P = 128
KD = D_IN // P           # 8 contraction subtiles for GEMM1
NH = D_HID // P          # 32 hidden tiles
NO = D_OUT // P          # 8 output row blocks (yT orientation)
GW = G * BT              # batch columns per group
NG = B // GW             # 8 groups

BF16 = mybir.dt.bfloat16
F32 = mybir.dt.float32
AF = mybir.ActivationFunctionType
ALU = mybir.AluOpType

nbf16 = ml_dtypes.bfloat16


def build_nc(batch=B, passes=1):
    assert batch % GW == 0
    ng = batch // GW

    nc = bacc.Bacc(trn_type="TRN2")

    xt_d = nc.dram_tensor("xt", [D_IN, batch], BF16, kind="ExternalInput")
    w1_d = nc.dram_tensor("w1r", [NH, P, KD, P], BF16, kind="ExternalInput")
    b1_d = nc.dram_tensor("b1c", [P, NH], F32, kind="ExternalInput")
    w2_d = nc.dram_tensor("w2", [D_HID, D_OUT], BF16, kind="ExternalInput")
    b2_d = nc.dram_tensor("b2c", [P, NO], F32, kind="ExternalInput")
    wg_d = nc.dram_tensor("wg", [KD, P, E], BF16, kind="ExternalInput")
    bg_d = nc.dram_tensor("bgc", [E, 1], F32, kind="ExternalInput")
    yt_d = nc.dram_tensor("yt", [D_OUT, batch], F32, kind="ExternalOutput")

    with tile.TileContext(nc) as tc:
        with (
            tc.tile_pool(name="const", bufs=1) as const,
            tc.tile_pool(name="w2pool", bufs=1) as w2pool,
            tc.tile_pool(name="w1pool", bufs=4) as w1pool,
            tc.tile_pool(name="xpool", bufs=24) as xpool,
            tc.tile_pool(name="hpool", bufs=68) as hpool,
            tc.tile_pool(name="gpool", bufs=3) as gpool,
            tc.tile_pool(name="gbcpool", bufs=4) as gbcpool,
            tc.tile_pool(name="ypool", bufs=4) as ypool,
            tc.tile_pool(name="ph", bufs=4, space="PSUM") as ph,
            tc.tile_pool(name="py", bufs=4, space="PSUM") as py,
        ):
            # ---- persistent tiles -------------------------------------
            wg_sb = const.tile([P, KD, E], BF16, tag="wg")
            nc.sync.dma_start(wg_sb[:], wg_d[:].rearrange("k p e -> p k e"))
            b1_sb = const.tile([P, NH], F32, tag="b1")
            nc.sync.dma_start(b1_sb[:], b1_d[:])
            b2_sb = const.tile([P, NO], F32, tag="b2")
            nc.sync.dma_start(b2_sb[:], b2_d[:])
            bg_sb = const.tile([E, 1], F32, tag="bg")
            nc.sync.dma_start(bg_sb[:], bg_d[:])

            # First group's x and first w1 blocks arrive before the bulk W2
            # load so the PE can start immediately.
            xts_first = []
            for g in range(G):
                for kd in range(KD):
                    t = xpool.tile([P, BT], BF16, tag="xt",
                                   name=f"xt0_{g}_{kd}")
                    nc.sync.dma_start(
                        t[:], xt_d[kd * P:(kd + 1) * P,
                                   g * BT:(g + 1) * BT])
                    xts_first.append(t)
            w1_first = []
            for nt in range(4):
                t = w1pool.tile([P, KD, P], BF16, tag="w1t",
                                name=f"w1t0_{nt}")
                nc.sync.dma_start(t[:], w1_d[nt])
                w1_first.append(t)

            w2_sb = []
            for kh in range(NH):
                t = w2pool.tile([P, D_OUT], BF16, tag=f"w2_{kh}")
                nc.sync.dma_start(t[:], w2_d[kh * P:(kh + 1) * P, :])
                w2_sb.append(t)

            # ---- main loop over batch groups --------------------------
            # passes>1 repeats the whole loop (same output) — used only by
            # the perf harness to measure device time as a wall-clock slope.
            for it, grp in enumerate(
                    [i for _ in range(passes) for i in range(ng)]):
                b0 = grp * GW

                if it == 0:
                    xts = [[xts_first[g * KD + kd] for kd in range(KD)]
                           for g in range(G)]
                else:
                    xts = []
                    for g in range(G):
                        row = []
                        for kd in range(KD):
                            t = xpool.tile([P, BT], BF16, tag="xt")
                            nc.sync.dma_start(
                                t[:], xt_d[kd * P:(kd + 1) * P,
                                           b0 + g * BT:b0 + (g + 1) * BT])
                            row.append(t)
                        xts.append(row)

                # gate: logitsT[e, b] with experts permuted so OUR expert is
                # row 0; exp on ScalarE; denominator via GPSIMD partition
                # all-reduce; gate row = exp[0] * 1/den; broadcast to 128
                # partitions on GPSIMD.  No PE involvement beyond the logits.
                gbcs = []
                lgs = [ph.tile([E, BT], F32, tag="acc", name=f"lg{g}")
                       for g in range(G)]
                for kd in range(KD):
                    # kd-outer / g-inner: the wg stationary is shared by the
                    # G batch tiles, like the main GEMM streams
                    for g in range(G):
                        nc.tensor.matmul(
                            lgs[g][:], lhsT=wg_sb[:, kd, :],
                            rhs=xts[g][kd][:],
                            start=(kd == 0), stop=(kd == KD - 1))
                for g in range(G):
                    lg = lgs[g]
                    exp_t = gpool.tile([E, BT], BF16, tag="exp")
                    nc.scalar.activation(exp_t[:], lg[:], AF.Exp,
                                         bias=bg_sb[:], scale=1.0)
                    sum8 = gpool.tile([E, BT], F32, tag="sum8")
                    nc.gpsimd.partition_all_reduce(sum8[:], exp_t[:], E,
                                                   ReduceOp.add)
                    rec = gpool.tile([1, BT], F32, tag="rec")
                    nc.vector.reciprocal(rec[:], sum8[0:1, :])
                    grow = gpool.tile([1, BT], BF16, tag="grow")
                    nc.vector.tensor_mul(out=grow[:], in0=exp_t[0:1, :],
                                         in1=rec[:])
                    gbc = gbcpool.tile([P, BT], BF16, tag="gbc")
                    nc.gpsimd.partition_broadcast(gbc[:], grow[:], P)
                    gbcs.append(gbc)

                # GEMM1: hT[n, b] = relu(sum_d W1[d,n] xT[d,b] + b1[n]),
                # W1 block stationary across the G batch tiles.
                hs = []
                for nt in range(NH):
                    if it == 0 and nt < 4:
                        w1t = w1_first[nt]
                    else:
                        w1t = w1pool.tile([P, KD, P], BF16, tag="w1t")
                        nc.sync.dma_start(w1t[:], w1_d[nt])
                    accs = [ph.tile([P, BT], F32, tag="acc", name=f"a{g}")
                            for g in range(G)]
                    for kd in range(KD):
                        for g in range(G):
                            nc.tensor.matmul(
                                accs[g][:], lhsT=w1t[:, kd, :],
                                rhs=xts[g][kd][:],
                                start=(kd == 0), stop=(kd == KD - 1))
                    row = []
                    for g in range(G):
                        h = hpool.tile([P, BT], BF16, tag="h")
                        nc.scalar.activation(h[:], accs[g][:], AF.Relu,
                                             bias=b1_sb[:, nt:nt + 1],
                                             scale=1.0)
                        row.append(h)
                    hs.append(row)

                # GEMM2: yT[o, b] = sum_h W2[h, o] hT[h, b]; epilogue fuses
                # evacuation with bias+gate: y = (psum + b2[o]) * gate[b].
                for ot in range(NO):
                    psy = [py.tile([P, BT], F32, tag="psy", name=f"p{g}")
                           for g in range(G)]
                    for kh in range(NH):
                        lhsT = w2_sb[kh][:, ot * P:(ot + 1) * P]
                        for g in range(G):
                            nc.tensor.matmul(
                                psy[g][:], lhsT=lhsT, rhs=hs[kh][g][:],
                                start=(kh == 0), stop=(kh == NH - 1))
                    for g in range(G):
                        yt = ypool.tile([P, BT], F32, tag="y")
                        nc.vector.scalar_tensor_tensor(
                            out=yt[:], in0=psy[g][:],
                            scalar=b2_sb[:, ot:ot + 1],
                            in1=gbcs[g][:],
                            op0=ALU.add, op1=ALU.mult)
                        nc.sync.dma_start(
                            yt_d[ot * P:(ot + 1) * P,
                                 b0 + g * BT:b0 + (g + 1) * BT],
                            yt[:])

    nc.finalize()
    return nc


def make_in_maps(x, W1, b1, W2, b2, Wg, bg, batch=B):
    """Host-side prep: transpose x once, cast matmul operands to bf16,
    pre-arrange W1 into [nt, p, kd, m] blocks, biases to on-chip layouts."""
    f32 = np.float32
    xt = np.ascontiguousarray(x.astype(f32).T).astype(nbf16)      # [D_IN, B]

    in_maps = []
    for e in range(NCORES):
        # permute experts so this core's expert is gate row 0
        perm = np.concatenate([[e], np.delete(np.arange(E), e)])
        wg = np.ascontiguousarray(
            Wg.astype(f32).reshape(KD, P, E)[:, :, perm]).astype(nbf16)
        bgc = np.ascontiguousarray(bg.astype(f32)[perm].reshape(E, 1))
        w1r = np.ascontiguousarray(
            W1[e].astype(f32).reshape(KD, P, NH, P).transpose(2, 1, 0, 3)
        ).astype(nbf16)                                   # [nt, p, kd, m]
        in_maps.append({
            "xt": xt,
            "w1r": w1r,
            "b1c": np.ascontiguousarray(
                b1[e].astype(f32).reshape(NH, P).T),
            "w2": np.ascontiguousarray(W2[e].astype(f32)).astype(nbf16),
            "b2c": np.ascontiguousarray(
                b2[e].astype(f32).reshape(NO, P).T),
            "wg": wg,
            "bgc": bgc,
        })
    return in_maps


def kernel(x, W1, b1, W2, b2, Wg, bg):
    in_maps = make_in_maps(x, W1, b1, W2, b2, Wg, bg)
    nc = build_nc(B)
    res = run_bass_kernel_spmd(nc, in_maps, core_ids=list(range(NCORES)))
    out = res.results[0]["yt"].astype(np.float64)
    for e in range(1, NCORES):
        out += res.results[e]["yt"]
    return np.ascontiguousarray(out.T).astype(np.float32)


# revision 20
# speedup vs baseline: 1.0555x; 1.0555x over previous
"""MoE layer (dense routing, 8 experts) on 8 TRN2 NeuronCores.

Expert-parallel: core e owns expert e; x and the gating network are
replicated.  Core e computes y_e^T = g_e * (relu(x W1[e] + b1) W2[e] + b2)^T
and the host sums the 8 per-core outputs and transposes once.

V2 layout: both GEMMs are weight-stationary and every stationary load is
shared by G=2 batch tiles (1024 batch columns in flight):

  - GEMM1 (transposed): hT[n, b] = relu(sum_d W1[d, n] xT[d, b] + b1[n]),
    stationary = W1 128x128 blocks (streamed from HBM per nt-block in a
    small rotating buffer), moving = xT batch tiles.  Each stationary block
    feeds the G batch tiles before switching.
  - GEMM2 (transposed output): yT[o, b] = sum_h W2[h, o] hT[h, b],
    stationary = W2 128x128 blocks (fully resident), moving = the hT tiles
    produced by GEMM1.  Epilogue fuses PSUM evacuation with bias and gate:
    y_sbuf = (psum + b2[o]) * gate_bcast[b].  Output stays transposed in
    DRAM; the host untransposes after the cross-core sum.

  - Gate: logitsT[e, b] for all 8 experts via 8 small matmuls, with the
    expert axis permuted per core so the own expert is row 0; exp on
    ScalarE with fused bg bias; softmax denominator via GPSIMD partition
    all-reduce; gate row = exp[0] * 1/den (DVE); broadcast to 128
    partitions on GPSIMD.  Beyond the logits the gate never touches the
    PE (no transposes, no num/den/broadcast matmuls).

All matmuls bf16 x bf16 -> fp32 PSUM.  W2 lives in SBUF for the whole
kernel; W1 streams per (group, nt) block; x streams in 512-column tiles.
"""

import numpy as np
import ml_dtypes

import concourse.bacc as bacc
import concourse.mybir as mybir
import concourse.tile as tile
from concourse.bass_isa import ReduceOp
from concourse.bass_utils import run_bass_kernel_spmd

B, D_IN, D_HID, D_OUT, E = 8192, 1024, 4096, 1024, 8
NCORES = 8
BT = 512                 # batch tile (matmul moving free dim)
G = 2                    # batch tiles sharing each stationary load
P = 128
KD = D_IN // P           # 8 contraction subtiles for GEMM1
NH = D_HID // P          # 32 hidden tiles
NO = D_OUT // P          # 8 output row blocks (yT orientation)
GW = G * BT              # batch columns per group
NG = B // GW             # 8 groups

BF16 = mybir.dt.bfloat16
F32 = mybir.dt.float32
AF = mybir.ActivationFunctionType
ALU = mybir.AluOpType

nbf16 = ml_dtypes.bfloat16


def build_nc(batch=B, passes=1):
    assert batch % GW == 0
    ng = batch // GW

    nc = bacc.Bacc(trn_type="TRN2")

    xt_d = nc.dram_tensor("xt", [D_IN, batch], BF16, kind="ExternalInput")
    w1_d = nc.dram_tensor("w1r", [NH, P, KD, P], BF16, kind="ExternalInput")
    b1_d = nc.dram_tensor("b1c", [P, NH], F32, kind="ExternalInput")
    w2_d = nc.dram_tensor("w2", [D_HID, D_OUT], BF16, kind="ExternalInput")
    b2_d = nc.dram_tensor("b2c", [P, NO], F32, kind="ExternalInput")
    wg_d = nc.dram_tensor("wg", [KD, P, E], BF16, kind="ExternalInput")
    bg_d = nc.dram_tensor("bgc", [E, 1], F32, kind="ExternalInput")
    yt_d = nc.dram_tensor("yt", [D_OUT, batch], F32, kind="ExternalOutput")

    with tile.TileContext(nc) as tc:
        with (
            tc.tile_pool(name="const", bufs=1) as const,
            tc.tile_pool(name="w2pool", bufs=1) as w2pool,
            tc.tile_pool(name="w1pool", bufs=4) as w1pool,
            tc.tile_pool(name="xpool", bufs=24) as xpool,
            tc.tile_pool(name="hpool", bufs=68) as hpool,
            tc.tile_pool(name="gpool", bufs=3) as gpool,
            tc.tile_pool(name="gbcpool", bufs=4) as gbcpool,
            tc.tile_pool(name="ypool", bufs=4) as ypool,
            tc.tile_pool(name="ph", bufs=4, space="PSUM") as ph,
            tc.tile_pool(name="py", bufs=4, space="PSUM") as py,
        ):
            # ---- persistent tiles -------------------------------------
            wg_sb = const.tile([P, KD, E], BF16, tag="wg")
            nc.sync.dma_start(wg_sb[:], wg_d[:].rearrange("k p e -> p k e"))
            b1_sb = const.tile([P, NH], F32, tag="b1")
            nc.sync.dma_start(b1_sb[:], b1_d[:])
            b2_sb = const.tile([P, NO], F32, tag="b2")
            nc.sync.dma_start(b2_sb[:], b2_d[:])
            bg_sb = const.tile([E, 1], F32, tag="bg")
            nc.sync.dma_start(bg_sb[:], bg_d[:])

            # First group's x and first w1 blocks arrive before the bulk W2
            # load so the PE can start immediately.
            xts_first = []
            for g in range(G):
                for kd in range(KD):
                    t = xpool.tile([P, BT], BF16, tag="xt",
                                   name=f"xt0_{g}_{kd}")
                    nc.sync.dma_start(
                        t[:], xt_d[kd * P:(kd + 1) * P,
                                   g * BT:(g + 1) * BT])
                    xts_first.append(t)
            w1_first = []
            for nt in range(4):
                t = w1pool.tile([P, KD, P], BF16, tag="w1t",
                                name=f"w1t0_{nt}")
                nc.sync.dma_start(t[:], w1_d[nt])
                w1_first.append(t)

            w2_sb = []
            for kh in range(NH):
                t = w2pool.tile([P, D_OUT], BF16, tag=f"w2_{kh}")
                nc.sync.dma_start(t[:], w2_d[kh * P:(kh + 1) * P, :])
                w2_sb.append(t)

            # ---- main loop over batch groups --------------------------
            # passes>1 repeats the whole loop (same output) — used only by
            # the perf harness to measure device time as a wall-clock slope.
            for it, grp in enumerate(
                    [i for _ in range(passes) for i in range(ng)]):
                b0 = grp * GW

                if it == 0:
                    xts = [[xts_first[g * KD + kd] for kd in range(KD)]
                           for g in range(G)]
                else:
                    xts = []
                    for g in range(G):
                        row = []
                        for kd in range(KD):
                            t = xpool.tile([P, BT], BF16, tag="xt")
                            nc.sync.dma_start(
                                t[:], xt_d[kd * P:(kd + 1) * P,
                                           b0 + g * BT:b0 + (g + 1) * BT])
                            row.append(t)
                        xts.append(row)

                # gate: logitsT[e, b] with experts permuted so OUR expert is
                # row 0; exp on ScalarE; denominator via GPSIMD partition
                # all-reduce; gate row = exp[0] * 1/den; broadcast to 128
                # partitions on GPSIMD.  No PE involvement beyond the logits.
                gbcs = []
                lgs = [ph.tile([E, BT], F32, tag="acc", name=f"lg{g}")
                       for g in range(G)]
                for kd in range(KD):
                    # kd-outer / g-inner: the wg stationary is shared by the
                    # G batch tiles, like the main GEMM streams
                    for g in range(G):
                        nc.tensor.matmul(
                            lgs[g][:], lhsT=wg_sb[:, kd, :],
                            rhs=xts[g][kd][:],
                            start=(kd == 0), stop=(kd == KD - 1))
                for g in range(G):
                    lg = lgs[g]
                    exp_t = gpool.tile([E, BT], BF16, tag="exp")
                    nc.scalar.activation(exp_t[:], lg[:], AF.Exp,
                                         bias=bg_sb[:], scale=1.0)
                    sum8 = gpool.tile([E, BT], F32, tag="sum8")
                    nc.gpsimd.partition_all_reduce(sum8[:], exp_t[:], E,
                                                   ReduceOp.add)
                    rec = gpool.tile([1, BT], F32, tag="rec")
                    nc.vector.reciprocal(rec[:], sum8[0:1, :])
                    grow = gpool.tile([1, BT], BF16, tag="grow")
                    nc.vector.tensor_mul(out=grow[:], in0=exp_t[0:1, :],
                                         in1=rec[:])
                    gbc = gbcpool.tile([P, BT], BF16, tag="gbc")
                    nc.gpsimd.partition_broadcast(gbc[:], grow[:], P)
                    gbcs.append(gbc)

                # GEMM1: hT[n, b] = relu(sum_d W1[d,n] xT[d,b] + b1[n]),
                # W1 block stationary across the G batch tiles.
                hs = []
                for nt in range(NH):
                    if it == 0 and nt < 4:
                        w1t = w1_first[nt]
                    else:
                        w1t = w1pool.tile([P, KD, P], BF16, tag="w1t")
                        nc.sync.dma_start(w1t[:], w1_d[nt])
                    accs = [ph.tile([P, BT], F32, tag="acc", name=f"a{g}")
                            for g in range(G)]
                    for kd in range(KD):
                        for g in range(G):
                            nc.tensor.matmul(
                                accs[g][:], lhsT=w1t[:, kd, :],
                                rhs=xts[g][kd][:],
                                start=(kd == 0), stop=(kd == KD - 1))
                    row = []
                    for g in range(G):
                        h = hpool.tile([P, BT], BF16, tag="h")
                        nc.scalar.activation(h[:], accs[g][:], AF.Relu,
                                             bias=b1_sb[:, nt:nt + 1],
                                             scale=1.0)
                        row.append(h)
                    hs.append(row)

                # GEMM2: yT[o, b] = sum_h W2[h, o] hT[h, b]; epilogue fuses
                # evacuation with bias+gate: y = (psum + b2[o]) * gate[b].
                for ot in range(NO):
                    psy = [py.tile([P, BT], F32, tag="psy", name=f"p{g}")
                           for g in range(G)]
                    for kh in range(NH):
                        lhsT = w2_sb[kh][:, ot * P:(ot + 1) * P]
                        for g in range(G):
                            nc.tensor.matmul(
                                psy[g][:], lhsT=lhsT, rhs=hs[kh][g][:],
                                start=(kh == 0), stop=(kh == NH - 1))
                    for g in range(G):
                        yt = ypool.tile([P, BT], F32, tag="y")
                        nc.vector.scalar_tensor_tensor(
                            out=yt[:], in0=psy[g][:],
                            scalar=b2_sb[:, ot:ot + 1],
                            in1=gbcs[g][:],
                            op0=ALU.add, op1=ALU.mult)
                        nc.sync.dma_start(
                            yt_d[ot * P:(ot + 1) * P,
                                 b0 + g * BT:b0 + (g + 1) * BT],
                            yt[:])

    nc.finalize()
    return nc


def make_in_maps(x, W1, b1, W2, b2, Wg, bg, batch=B):
    """Host-side prep: transpose x once, cast matmul operands to bf16,
    pre-arrange W1 into [nt, p, kd, m] blocks, biases to on-chip layouts."""
    f32 = np.float32
    xt = np.ascontiguousarray(x.astype(f32).T).astype(nbf16)      # [D_IN, B]

    in_maps = []
    for e in range(NCORES):
        # permute experts so this core's expert is gate row 0
        perm = np.concatenate([[e], np.delete(np.arange(E), e)])
        wg = np.ascontiguousarray(
            Wg.astype(f32).reshape(KD, P, E)[:, :, perm]).astype(nbf16)
        bgc = np.ascontiguousarray(bg.astype(f32)[perm].reshape(E, 1))
        w1r = np.ascontiguousarray(
            W1[e].astype(f32).reshape(KD, P, NH, P).transpose(2, 1, 0, 3)
        ).astype(nbf16)                                   # [nt, p, kd, m]
        in_maps.append({
            "xt": xt,
            "w1r": w1r,
            "b1c": np.ascontiguousarray(
                b1[e].astype(f32).reshape(NH, P).T),
            "w2": np.ascontiguousarray(W2[e].astype(f32)).astype(nbf16),
            "b2c": np.ascontiguousarray(
                b2[e].astype(f32).reshape(NO, P).T),
            "wg": wg,
            "bgc": bgc,
        })
    return in_maps


def kernel(x, W1, b1, W2, b2, Wg, bg):
    in_maps = make_in_maps(x, W1, b1, W2, b2, Wg, bg)
    nc = build_nc(B)
    res = run_bass_kernel_spmd(nc, in_maps, core_ids=list(range(NCORES)))
    out = res.results[0]["yt"].astype(np.float64)
    for e in range(1, NCORES):
        out += res.results[e]["yt"]
    return np.ascontiguousarray(out.T).astype(np.float32)
